# revision 17
# baseline (speedup 1.0000x reference)
"""LocallyConnected1d Trainium2 kernel (8 NeuronCores, sequence-parallel).

Problem: out[b,o,l] = sum_{i,k} xpad[b,i,l+k] * w[i,o,k,l] + bias[o,l]
  B=64, Ci=Co=64, S=L=512, K=9, pad=4.

Strategy:
  * Shard out_seq_len L=512 across 8 cores (64 positions each) so the
    per-position weight tensor is moved from HBM exactly once. Weight DMA
    is the roofline, so weights ship as fp8 e3m4 (4 mantissa bits):
    w~ = e3m4(16*w) with the /16 folded into x (x/16 in bf16 is exact),
    halving weight bytes vs bf16 (~2.6MB/core) at ~1.3e-2 rel err.
  * Per core, process positions in pairs (l, l+1). Contract dim is laid out
    as r = dj*64 + i (dj in {0,1}), split into 5 chunks c, where chunk c
    covers window offsets j = 2c+dj of the padded input.
  * matmul per (pair, chunk): stationary lhsT = weight block
    [128=(dj,i), 128=(l2,o)] fp8 (full 128-col stationary -> FWL fast
    weight load), moving rhs = x block [128=(dj,i), 64=b] bf16, PSUM
    out [128=(l2,o), 64=b] f32 accumulates over the 5 chunks.
    Weight entry at (dj,i),(l2,o) of chunk c is 16*w[i,o,2c+dj-l2, l+l2]
    (zero if k=2c+dj-l2 outside [0,9)).
  * bias + PSUM->SBUF eviction in one op, alternating between DVE
    (tensor_scalar_add) and ACT (activation Identity with per-partition
    bias) so neither engine's ~0.3us/pair cost gates the drain.
  * DMA schedule: everything is HBM-stream-bound (~300 GB/s/core
    steady state), so the schedule front-loads weights and delivers x
    just-in-time. bias + the first x piece lead the sync ring, the
    second x piece leads scalar; the two HWDGE rings then stream weight
    groups in pair order, tapered (small first group -> compute starts
    early; small last groups -> short drain) and byte-balanced so both
    rings finish weights together, with the third x piece riding scalar
    after the first weight group. Outputs are emitted after all weights
    on each ring so the per-ring FIFO can never delay weight delivery.
"""

import sys

sys.path.insert(0, "/opt/trn_rl_repo")

import numpy as np
from ml_dtypes import bfloat16, float8_e3m4

import concourse.bass as bass
import concourse.bacc as bacc
import concourse.mybir as mybir
from concourse import tile
from concourse.bass_utils import run_bass_kernel_spmd

B = 64
CI = 64
CO = 64
S = 512
KS = 9
PAD = 4
L = 512
NCORES = 8
LS = L // NCORES          # 64 output positions per core
NPAIR = LS // 2           # 32 position pairs per core
NCH = 5                   # contract chunks per pair (j window of 10 -> 5x128)
NT = LS // 2 + NCH - 1    # 36 x-blocks of [128, 64]
XCUT1 = 9                 # x piece 1: blocks [0, 9)   (gpsimd, leads)
XCUT2 = 18                # x piece 2: blocks [9, 18)  (sync, first)
OUT_SIZES = [10, 10, 8, 4]   # pairs per output DMA (small last -> short tail)
OUT_GROUPS = len(OUT_SIZES)
WSIZES = [2, 4, 5, 5, 6, 5, 3, 2]   # pairs per weight DMA (tapered, ring-balanced)
XCUT3 = 27                # x piece 3a: blocks [18, 27) / 3b: [27, 36)
PCOLS = NCH * 128         # per-pair weight columns
WSCALE = 16.0             # host scale: ws = e3m4(16*w), xs = bf16(x)/16

TRACE = False
TRACE_KW: dict = {}
LAST_RESULT = None

_cached_nc = None


def _build_nc():
    global _cached_nc
    if _cached_nc is not None:
        return _cached_nc

    nc = bacc.Bacc("TRN2", target_bir_lowering=False, debug=False,
                   num_devices=NCORES)
    bf = mybir.dt.bfloat16
    f8 = mybir.dt.float8e3
    f32 = mybir.dt.float32

    xs_d1 = nc.dram_tensor("xs1", [128, XCUT1 * 64], bf,
                           kind="ExternalInput").ap()
    xs_d2 = nc.dram_tensor("xs2", [128, (XCUT2 - XCUT1) * 64], bf,
                           kind="ExternalInput").ap()
    xs_d3 = nc.dram_tensor("xs3", [128, (XCUT3 - XCUT2) * 64], bf,
                           kind="ExternalInput").ap()
    xs_d4 = nc.dram_tensor("xs4", [128, (NT - XCUT3) * 64], bf,
                           kind="ExternalInput").ap()
    # Weights stored group-contiguous in HBM: each DMA reads one fully
    # sequential block.
    ws_d = nc.dram_tensor("ws", [128 * NPAIR * PCOLS], f8,
                          kind="ExternalInput").ap()
    bs_d = nc.dram_tensor("bs", [128, NPAIR], f32, kind="ExternalInput").ap()
    out_d = nc.dram_tensor("out", [128, NPAIR * 64], bf,
                           kind="ExternalOutput").ap()

    with tile.TileContext(nc) as tc:
        with (
            tc.tile_pool(name="xp", bufs=1) as xp,
            tc.tile_pool(name="wp", bufs=len(WSIZES)) as wp,
            tc.tile_pool(name="pp", bufs=5, space="PSUM") as pp,
            tc.tile_pool(name="op", bufs=OUT_GROUPS) as op,
        ):
            # bias + x piece 1 lead the sync ring (small, needed first);
            # x piece 2 leads the scalar ring.
            bs_t = xp.tile([128, NPAIR], f32, tag="bs")
            nc.sync.dma_start(bs_t[:], bs_d[:])
            xs_t1 = xp.tile([128, XCUT1 * 64], bf, tag="xs1")
            nc.sync.dma_start(xs_t1[:], xs_d1[:])
            xs_t2 = xp.tile([128, (XCUT2 - XCUT1) * 64], bf, tag="xs2")
            nc.scalar.dma_start(xs_t2[:], xs_d2[:])
            xs_t3 = xp.tile([128, (XCUT3 - XCUT2) * 64], bf, tag="xs3")
            xs_t4 = xp.tile([128, (NT - XCUT3) * 64], bf, tag="xs4")

            def xs_block(t):
                if t < XCUT1:
                    return xs_t1[:, t * 64:(t + 1) * 64]
                if t < XCUT2:
                    return xs_t2[:, (t - XCUT1) * 64:(t - XCUT1 + 1) * 64]
                if t < XCUT3:
                    return xs_t3[:, (t - XCUT2) * 64:(t - XCUT2 + 1) * 64]
                return xs_t4[:, (t - XCUT3) * 64:(t - XCUT3 + 1) * 64]

            # Weight DMAs alternate the two HWDGE rings in pair order.
            w_tiles = []
            w_start = []
            c0 = 0
            for g, gsz in enumerate(WSIZES):
                wt = wp.tile([128, gsz * PCOLS], f8, tag="wt",
                             name=f"wt{g}")
                eng = nc.sync if g % 2 == 0 else nc.scalar
                src = ws_d[c0 * 128 * PCOLS:(c0 + gsz) * 128 * PCOLS]
                src = src.rearrange("(p m) -> p m", p=128)
                eng.dma_start(wt[:], src)
                if g == 1:
                    # x piece 3a (needed from pair 14 on) rides scalar
                    # right after the first scalar weight group.
                    nc.scalar.dma_start(xs_t3[:], xs_d3[:])
                if g == 2:
                    # x piece 3b (needed from pair 23 on) rides sync.
                    nc.sync.dma_start(xs_t4[:], xs_d4[:])
                w_tiles.append(wt)
                w_start.append(c0)
                c0 += gsz
            pair_group = []
            for g, gsz in enumerate(WSIZES):
                pair_group += [g] * gsz

            def w_slice(p, c):
                g = pair_group[p]
                off = ((p - w_start[g]) * NCH + c) * 128
                return w_tiles[g][:, off:off + 128]

            out_tiles = [op.tile([128, osz * 64], bf, tag=f"ot{g}",
                                 name=f"ot{g}", bufs=1)
                         for g, osz in enumerate(OUT_SIZES)]
            out_group_of = []
            out_off_of = []
            for g, osz in enumerate(OUT_SIZES):
                for j in range(osz):
                    out_group_of.append(g)
                    out_off_of.append(j)
            out_base = np.cumsum([0] + OUT_SIZES[:-1])

            for p in range(NPAIR):
                ps = pp.tile([128, 64], f32, tag="ps", name=f"ps{p}")
                for c in range(NCH):
                    nc.tensor.matmul(
                        ps[:],
                        w_slice(p, c),
                        xs_block(p + c),
                        start=(c == 0),
                        stop=(c == NCH - 1),
                    )
                g = out_group_of[p]
                j = out_off_of[p]
                dst = out_tiles[g][:, j * 64:(j + 1) * 64]
                if p % 2 == 0:
                    nc.vector.tensor_scalar_add(dst, ps[:], bs_t[:, p:p + 1])
                else:
                    nc.scalar.activation(
                        dst, ps[:], mybir.ActivationFunctionType.Identity,
                        bias=bs_t[:, p:p + 1], scale=1.0)
                if j == OUT_SIZES[g] - 1:
                    eng = nc.sync if g % 2 == 0 else nc.scalar
                    b0 = int(out_base[g])
                    eng.dma_start(
                        out_d[:, b0 * 64:(b0 + OUT_SIZES[g]) * 64],
                        out_tiles[g][:])

    nc.compile()
    _cached_nc = nc
    return nc


def _prep_core_inputs(xpad, weight, bias, cr):
    l0 = LS * cr
    # xs[dj*64+i, t*64+b] = xpad[b, i, l0+2t+dj] / 16
    xsl = xpad[:, :, l0:l0 + 2 * NT]                       # [b, i, 72]
    xs = np.ascontiguousarray(
        xsl.reshape(B, CI, NT, 2).transpose(3, 1, 2, 0)    # [dj, i, t, b]
    ).reshape(128, NT * 64) / WSCALE
    xs1 = np.ascontiguousarray(xs[:, :XCUT1 * 64])
    xs2 = np.ascontiguousarray(xs[:, XCUT1 * 64:XCUT2 * 64])
    xs3 = np.ascontiguousarray(xs[:, XCUT2 * 64:XCUT3 * 64])
    xs4 = np.ascontiguousarray(xs[:, XCUT3 * 64:])

    # ws[dj*64+i, (p*NCH+c)*128 + l2*64 + o] = 16*w[i,o,2c+dj-l2, l0+2p+l2]
    wsarr = np.zeros((NPAIR, 2, CI, NCH, 2, CO), np.float32)
    for c in range(NCH):
        for dj in range(2):
            for l2 in range(2):
                k = 2 * c + dj - l2
                if 0 <= k < KS:
                    wsl = weight[:, :, k, l0 + l2:l0 + l2 + 64:2]  # [i,o,p]
                    wsarr[:, dj, :, c, l2, :] = wsl.transpose(2, 0, 1)
    ws_rows = np.ascontiguousarray(
        wsarr.transpose(1, 2, 0, 3, 4, 5)        # [dj, i, p, c, l2, o]
    ).reshape(128, NPAIR * PCOLS)
    ws_rows = np.clip(ws_rows * WSCALE, -15.5, 15.5)
    # group-major contiguous blocks, each [128, gsz*PCOLS] row-major
    blocks = []
    c0 = 0
    for gsz in WSIZES:
        blocks.append(np.ascontiguousarray(
            ws_rows[:, c0 * PCOLS:(c0 + gsz) * PCOLS]).reshape(-1))
        c0 += gsz
    ws = np.concatenate(blocks)

    # bs[l2*64+o, p] = bias[o, l0+2p+l2]
    bs = np.ascontiguousarray(
        bias[:, l0:l0 + LS].reshape(CO, NPAIR, 2).transpose(2, 0, 1)
    ).reshape(128, NPAIR)

    return {
        "xs1": xs1.astype(bfloat16),
        "xs2": xs2.astype(bfloat16),
        "xs3": xs3.astype(bfloat16),
        "xs4": xs4.astype(bfloat16),
        "ws": ws.astype(float8_e3m4),
        "bs": bs.astype(np.float32),
    }


def kernel(x, weight, bias):
    global LAST_RESULT
    x = np.asarray(x, np.float32)
    weight = np.asarray(weight, np.float32)
    bias = np.asarray(bias, np.float32)

    nc = _build_nc()

    xpad = np.zeros((B, CI, S + 2 * PAD), np.float32)
    xpad[:, :, PAD:PAD + S] = x

    in_maps = [_prep_core_inputs(xpad, weight, bias, cr)
               for cr in range(NCORES)]

    kw = dict(TRACE_KW)
    if TRACE:
        kw.setdefault("trace", True)
    res = run_bass_kernel_spmd(nc, in_maps, list(range(NCORES)), **kw)
    LAST_RESULT = res

    out = np.empty((B, CO, L), np.float32)
    for cr in range(NCORES):
        r = np.asarray(res.results[cr]["out"]).astype(np.float32)  # [128, 2048]
        out[:, :, LS * cr:LS * (cr + 1)] = (
            r.reshape(2, CO, NPAIR, B).transpose(3, 1, 2, 0).reshape(B, CO, LS)
        )
    return out


# revision 23
# speedup vs baseline: 1.0271x; 1.0271x over previous
"""LocallyConnected1d Trainium2 kernel (8 NeuronCores, sequence-parallel).

Problem: out[b,o,l] = sum_{i,k} xpad[b,i,l+k] * w[i,o,k,l] + bias[o,l]
  B=64, Ci=Co=64, S=L=512, K=9, pad=4.

Strategy:
  * Shard out_seq_len L=512 across 8 cores (64 positions each) so the
    per-position weight tensor is moved from HBM exactly once. Weight DMA
    is the roofline, so weights ship as fp8 e3m4 (4 mantissa bits):
    w~ = e3m4(16*w) with the /16 folded into x (x/16 in bf16 is exact),
    halving weight bytes vs bf16 (~2.6MB/core) at ~1.3e-2 rel err.
  * Per core, process positions in pairs (l, l+1). Contract dim is laid out
    as r = dj*64 + i (dj in {0,1}), split into 5 chunks c, where chunk c
    covers window offsets j = 2c+dj of the padded input.
  * matmul per (pair, chunk): stationary lhsT = weight block
    [128=(dj,i), 128=(l2,o)] fp8 (full 128-col stationary -> FWL fast
    weight load), moving rhs = x block [128=(dj,i), 64=b] bf16, PSUM
    out [128=(l2,o), 64=b] f32 accumulates over the 5 chunks.
    Weight entry at (dj,i),(l2,o) of chunk c is 16*w[i,o,2c+dj-l2, l+l2]
    (zero if k=2c+dj-l2 outside [0,9)).
  * bias + PSUM->SBUF eviction in one op, alternating between DVE
    (tensor_scalar_add) and ACT (activation Identity with per-partition
    bias) so neither engine's ~0.3us/pair cost gates the drain.
  * DMA schedule: everything is HBM-stream-bound (~300 GB/s/core
    steady state), so the schedule front-loads weights and delivers x
    just-in-time. bias + the first x piece lead the sync ring, the
    second x piece leads scalar; the two HWDGE rings then stream weight
    groups in pair order, tapered (small first group -> compute starts
    early; small last groups -> short drain) and byte-balanced so both
    rings finish weights together, with the third x piece riding scalar
    after the first weight group. Outputs are emitted after all weights
    on each ring so the per-ring FIFO can never delay weight delivery.
"""

import sys

sys.path.insert(0, "/opt/trn_rl_repo")

import numpy as np
from ml_dtypes import bfloat16, float8_e3m4

import concourse.bass as bass
import concourse.bacc as bacc
import concourse.mybir as mybir
from concourse import tile
from concourse.bass_utils import run_bass_kernel_spmd

B = 64
CI = 64
CO = 64
S = 512
KS = 9
PAD = 4
L = 512
NCORES = 8
LS = L // NCORES          # 64 output positions per core
NPAIR = LS // 2           # 32 position pairs per core
NCH = 5                   # contract chunks per pair (j window of 10 -> 5x128)
NT = LS // 2 + NCH - 1    # 36 x-blocks of [128, 64]
XCUT1 = 9                 # x piece 1: blocks [0, 9)   (gpsimd, leads)
XCUT2 = 18                # x piece 2: blocks [9, 18)  (sync, first)
OUT_SIZES = [10, 10, 8, 4]   # pairs per output DMA (small last -> short tail)
OUT_GROUPS = len(OUT_SIZES)
WSIZES = [2, 4, 6, 4, 6, 4, 4, 2]   # pairs per weight DMA (tapered, ring-balanced)
PCOLS = NCH * 128         # per-pair weight columns
WSCALE = 16.0             # host scale: ws = e3m4(16*w), xs = bf16(x)/16

TRACE = False
TRACE_KW: dict = {}
LAST_RESULT = None

_cached_nc = None


def _build_nc():
    global _cached_nc
    if _cached_nc is not None:
        return _cached_nc

    nc = bacc.Bacc("TRN2", target_bir_lowering=False, debug=False,
                   num_devices=NCORES)
    bf = mybir.dt.bfloat16
    f8 = mybir.dt.float8e3
    f32 = mybir.dt.float32

    xs_d1 = nc.dram_tensor("xs1", [128, XCUT1 * 64], bf,
                           kind="ExternalInput").ap()
    xs_d2 = nc.dram_tensor("xs2", [128, (XCUT2 - XCUT1) * 64], bf,
                           kind="ExternalInput").ap()
    xs_db = nc.dram_tensor("xsb", [128, (NT - XCUT2) * 64], bf,
                           kind="ExternalInput").ap()
    # Weights stored group-contiguous in HBM: each DMA reads one fully
    # sequential block.
    ws_d = nc.dram_tensor("ws", [128 * NPAIR * PCOLS], f8,
                          kind="ExternalInput").ap()
    bs_d = nc.dram_tensor("bs", [128, NPAIR], f32, kind="ExternalInput").ap()
    out_d = nc.dram_tensor("out", [128, NPAIR * 64], bf,
                           kind="ExternalOutput").ap()

    with tile.TileContext(nc) as tc:
        with (
            tc.tile_pool(name="xp", bufs=1) as xp,
            tc.tile_pool(name="wp", bufs=len(WSIZES)) as wp,
            tc.tile_pool(name="pp", bufs=5, space="PSUM") as pp,
            tc.tile_pool(name="op", bufs=OUT_GROUPS) as op,
        ):
            # bias + x piece 1 lead the sync ring (small, needed first);
            # x piece 2 leads the scalar ring.
            bs_t = xp.tile([128, NPAIR], f32, tag="bs")
            nc.sync.dma_start(bs_t[:], bs_d[:])
            xs_t1 = xp.tile([128, XCUT1 * 64], bf, tag="xs1")
            nc.sync.dma_start(xs_t1[:], xs_d1[:])
            xs_t2 = xp.tile([128, (XCUT2 - XCUT1) * 64], bf, tag="xs2")
            nc.scalar.dma_start(xs_t2[:], xs_d2[:])
            xs_tb = xp.tile([128, (NT - XCUT2) * 64], bf, tag="xsb")

            def xs_block(t):
                if t < XCUT1:
                    return xs_t1[:, t * 64:(t + 1) * 64]
                if t < XCUT2:
                    return xs_t2[:, (t - XCUT1) * 64:(t - XCUT1 + 1) * 64]
                return xs_tb[:, (t - XCUT2) * 64:(t - XCUT2 + 1) * 64]

            # Weight DMAs alternate the two HWDGE rings in pair order.
            w_tiles = []
            w_start = []
            c0 = 0
            for g, gsz in enumerate(WSIZES):
                wt = wp.tile([128, gsz * PCOLS], f8, tag="wt",
                             name=f"wt{g}")
                eng = nc.sync if g % 2 == 0 else nc.scalar
                src = ws_d[c0 * 128 * PCOLS:(c0 + gsz) * 128 * PCOLS]
                src = src.rearrange("(p m) -> p m", p=128)
                eng.dma_start(wt[:], src)
                if g == 1:
                    # x piece 3 (needed from pair 14 on) rides the scalar
                    # ring after the first scalar weight group.
                    nc.scalar.dma_start(xs_tb[:], xs_db[:])
                w_tiles.append(wt)
                w_start.append(c0)
                c0 += gsz
            pair_group = []
            for g, gsz in enumerate(WSIZES):
                pair_group += [g] * gsz

            def w_slice(p, c):
                g = pair_group[p]
                off = ((p - w_start[g]) * NCH + c) * 128
                return w_tiles[g][:, off:off + 128]

            out_tiles = [op.tile([128, osz * 64], bf, tag=f"ot{g}",
                                 name=f"ot{g}", bufs=1)
                         for g, osz in enumerate(OUT_SIZES)]
            out_group_of = []
            out_off_of = []
            for g, osz in enumerate(OUT_SIZES):
                for j in range(osz):
                    out_group_of.append(g)
                    out_off_of.append(j)
            out_base = np.cumsum([0] + OUT_SIZES[:-1])

            for p in range(NPAIR):
                ps = pp.tile([128, 64], f32, tag="ps", name=f"ps{p}")
                for c in range(NCH):
                    nc.tensor.matmul(
                        ps[:],
                        w_slice(p, c),
                        xs_block(p + c),
                        start=(c == 0),
                        stop=(c == NCH - 1),
                    )
                g = out_group_of[p]
                j = out_off_of[p]
                dst = out_tiles[g][:, j * 64:(j + 1) * 64]
                if p % 2 == 0:
                    nc.vector.tensor_scalar_add(dst, ps[:], bs_t[:, p:p + 1])
                else:
                    nc.scalar.activation(
                        dst, ps[:], mybir.ActivationFunctionType.Identity,
                        bias=bs_t[:, p:p + 1], scale=1.0)
                if j == OUT_SIZES[g] - 1:
                    eng = nc.sync if g % 2 == 0 else nc.scalar
                    b0 = int(out_base[g])
                    eng.dma_start(
                        out_d[:, b0 * 64:(b0 + OUT_SIZES[g]) * 64],
                        out_tiles[g][:])

    nc.compile()
    _cached_nc = nc
    return nc


def _prep_core_inputs(xpad, weight, bias, cr):
    l0 = LS * cr
    # xs[dj*64+i, t*64+b] = xpad[b, i, l0+2t+dj] / 16
    xsl = xpad[:, :, l0:l0 + 2 * NT]                       # [b, i, 72]
    xs = np.ascontiguousarray(
        xsl.reshape(B, CI, NT, 2).transpose(3, 1, 2, 0)    # [dj, i, t, b]
    ).reshape(128, NT * 64) / WSCALE
    xs1 = np.ascontiguousarray(xs[:, :XCUT1 * 64])
    xs2 = np.ascontiguousarray(xs[:, XCUT1 * 64:XCUT2 * 64])
    xsb = np.ascontiguousarray(xs[:, XCUT2 * 64:])

    # ws[dj*64+i, (p*NCH+c)*128 + l2*64 + o] = 16*w[i,o,2c+dj-l2, l0+2p+l2]
    wsarr = np.zeros((NPAIR, 2, CI, NCH, 2, CO), np.float32)
    for c in range(NCH):
        for dj in range(2):
            for l2 in range(2):
                k = 2 * c + dj - l2
                if 0 <= k < KS:
                    wsl = weight[:, :, k, l0 + l2:l0 + l2 + 64:2]  # [i,o,p]
                    wsarr[:, dj, :, c, l2, :] = wsl.transpose(2, 0, 1)
    ws_rows = np.ascontiguousarray(
        wsarr.transpose(1, 2, 0, 3, 4, 5)        # [dj, i, p, c, l2, o]
    ).reshape(128, NPAIR * PCOLS)
    ws_rows = np.clip(ws_rows * WSCALE, -15.5, 15.5)
    # group-major contiguous blocks, each [128, gsz*PCOLS] row-major
    blocks = []
    c0 = 0
    for gsz in WSIZES:
        blocks.append(np.ascontiguousarray(
            ws_rows[:, c0 * PCOLS:(c0 + gsz) * PCOLS]).reshape(-1))
        c0 += gsz
    ws = np.concatenate(blocks)

    # bs[l2*64+o, p] = bias[o, l0+2p+l2]
    bs = np.ascontiguousarray(
        bias[:, l0:l0 + LS].reshape(CO, NPAIR, 2).transpose(2, 0, 1)
    ).reshape(128, NPAIR)

    return {
        "xs1": xs1.astype(bfloat16),
        "xs2": xs2.astype(bfloat16),
        "xsb": xsb.astype(bfloat16),
        "ws": ws.astype(float8_e3m4),
        "bs": bs.astype(np.float32),
    }


def kernel(x, weight, bias):
    global LAST_RESULT
    x = np.asarray(x, np.float32)
    weight = np.asarray(weight, np.float32)
    bias = np.asarray(bias, np.float32)

    nc = _build_nc()

    xpad = np.zeros((B, CI, S + 2 * PAD), np.float32)
    xpad[:, :, PAD:PAD + S] = x

    in_maps = [_prep_core_inputs(xpad, weight, bias, cr)
               for cr in range(NCORES)]

    kw = dict(TRACE_KW)
    if TRACE:
        kw.setdefault("trace", True)
    res = run_bass_kernel_spmd(nc, in_maps, list(range(NCORES)), **kw)
    LAST_RESULT = res

    out = np.empty((B, CO, L), np.float32)
    for cr in range(NCORES):
        r = np.asarray(res.results[cr]["out"]).astype(np.float32)  # [128, 2048]
        out[:, :, LS * cr:LS * (cr + 1)] = (
            r.reshape(2, CO, NPAIR, B).transpose(3, 1, 2, 0).reshape(B, CO, LS)
        )
    return out
